# revision 12
# baseline (speedup 1.0000x reference)
"""Expert-parallel MoE conditional feed-forward for 8 Trainium2 NeuronCores.

Problem: x[16,1024], expert_indices[16,2], gate/down_proj[8,2816,1024],
up_proj[8,1024,2816]. Reference computes, per (token, slot) pair with
e = expert_indices[t, a]:
    out[t,a,:] = (silu(x @ gate_proj[e].T) * (x @ down_proj[e].T)) @ up_proj[e].T

Sharding: core k owns expert k and computes its FFN output for ALL 16
tokens (the compute is negligible; the kernel is weight-streaming bound).
The host then gathers rows per expert_indices. This needs no indices on
device and is load-balanced regardless of routing.

Device kernel (per core): loop over 11 chunks of 256 of the 2816-wide
intermediate dim. Weights are host-packed into W[11, 128, 6144] so each
chunk is two contiguous DMAs (gate|down 2 MB, up 1 MB):
    W[c,p, hc*512+o]        o<256: gate block g[c*256+o, hc*128+p]
                            o>=256: down block d[c*256+o-256, hc*128+p]
    W[c,p, 4096+f*1024+j]   up block u[j, c*256+f*128+p]
All big matmuls stream the WEIGHT as the moving operand (the stationary
is a 16-column token tile), so there are no 128-column fp32 LDWEIGHTS.

fp32 matmuls cost 4 cycles/column on the PE, which would make the PE the
bottleneck (~113us) over the ~95us HBM roofline. Since only 16 of 128
array columns are used (M = 16 tokens), we run THREE concurrent column
quarter-strips (array packing, tile inferred from PSUM base partition;
q3 is unusable per HW bug):
    q1 (psum rows 32-47): gate|down chains for even chunks
    q2 (psum rows 64-79): gate|down chains for odd chunks
    q0 (psum rows  0-15): all up-projection accumulation into psum_out
Chunks are processed in pairs; the previous pair's 8 up-matmuls are
round-robin interleaved with the current pair's 2x8 chain matmuls so
consecutive PE instructions hit different strips and overlap (~2.8x
measured). The [16,128] intermediates are transposed to [128,16] via
identity matmuls on the PE (full-array mode), then fed as stationaries.
"""

import sys

for _p in ("/opt/trn_rl_repo", "/opt/pypackages"):
    if _p not in sys.path:
        sys.path.append(_p)

import numpy as np

NUM_EXPERTS = 8
HIDDEN = 1024
INTER = 2816
T = 16
N_CORES = 8
P = 128
CW = 256                  # intermediate chunk width
NCHUNK = INTER // CW      # 11
HC = HIDDEN // P          # 8 hidden chunks
U_OFF = 2 * HC * CW       # 4096: offset of up blocks in packed W
WCOLS = U_OFF + 2 * HIDDEN  # 6144

_COMPILED = None
LAST_RESULTS = None
TRACE = False


def _build():
    import concourse.bacc as bacc
    import concourse.bass as bass
    import concourse.tile as tile
    from concourse import mybir

    f32 = mybir.dt.float32
    nc = bacc.Bacc("TRN2", target_bir_lowering=False, debug=False,
                   num_devices=N_CORES)
    xt_d = nc.dram_tensor("xt", [P, HC * T], f32, kind="ExternalInput")
    eye_d = nc.dram_tensor("eye", [T, T], f32, kind="ExternalInput")
    w_d = nc.dram_tensor("w", [NCHUNK, P, WCOLS], f32, kind="ExternalInput")
    out_d = nc.dram_tensor("out", [T, HIDDEN], f32, kind="ExternalOutput")

    with tile.TileContext(nc) as tc:
        with (
            tc.tile_pool(name="xp", bufs=1) as xp,
            tc.tile_pool(name="wp", bufs=5) as wp,
            tc.tile_pool(name="ip", bufs=4) as ip,
            tc.tile_pool(name="pg", bufs=4, space=bass.MemorySpace.PSUM) as pgp,
            tc.tile_pool(name="tp", bufs=2, space=bass.MemorySpace.PSUM) as tpp,
            tc.tile_pool(name="po", bufs=1, space=bass.MemorySpace.PSUM) as pop,
            tc.tile_pool(name="op", bufs=1) as op,
        ):
            xt = xp.tile([P, HC * T], f32)
            nc.gpsimd.dma_start(xt[:], xt_d.ap())
            eye = xp.tile([T, T], f32)
            nc.gpsimd.dma_start(eye[:], eye_d.ap())

            psum_out = pop.tile([T, HIDDEN], f32)
            mm3_count = [0, 0]   # per-jb position in the accumulation chain
            pending_mm3 = []     # thunks deferred from the previous pair

            def emit_chunk_tail(c, w_u, pgd, base):
                """silu/mul + transposes for chunk c; queue its 4 up-matmuls."""
                s1 = ip.tile([T, CW], f32)
                nc.scalar.activation(s1[:], pgd[base:base + T, 0:CW],
                                     mybir.ActivationFunctionType.Silu)
                inter = ip.tile([T, CW], f32)
                nc.vector.tensor_mul(inter[:], s1[:],
                                     pgd[base:base + T, CW:2 * CW])
                for f in range(CW // P):
                    tp = tpp.tile([P, T], f32)
                    nc.tensor.matmul(tp[:], inter[:, f * P:(f + 1) * P], eye[:])
                    it = ip.tile([P, T], f32)
                    nc.vector.tensor_copy(it[:], tp[:])
                    for jb in range(HIDDEN // 512):
                        def mm3(it=it, w_u=w_u, f=f, jb=jb):
                            k = mm3_count[jb]
                            mm3_count[jb] += 1
                            nc.tensor.matmul(
                                psum_out[:, jb * 512:(jb + 1) * 512], it[:],
                                w_u[:, f * HIDDEN + jb * 512:
                                    f * HIDDEN + (jb + 1) * 512],
                                start=(k == 0), stop=(k == 2 * NCHUNK - 1),
                            )
                        pending_mm3.append(mm3)

            HGD = U_OFF // 2  # 2048 cols = hc 0-3 in one tile, hc 4-7 in other
            for c0 in range(0, NCHUNK, 2):
                pair = [c0] + ([c0 + 1] if c0 + 1 < NCHUNK else [])
                tiles = []
                for c, base in zip(pair, (32, 64)):
                    w_gd1 = wp.tile([P, HGD], f32, tag="wgd1")
                    nc.sync.dma_start(w_gd1[:], w_d.ap()[c][:, 0:HGD])
                    w_gd2 = wp.tile([P, HGD], f32, tag="wgd2")
                    nc.sync.dma_start(w_gd2[:], w_d.ap()[c][:, HGD:U_OFF])
                    w_u = wp.tile([P, WCOLS - U_OFF], f32, tag="wu")
                    nc.sync.dma_start(w_u[:], w_d.ap()[c][:, U_OFF:WCOLS])
                    pgd = pgp.tile([P, 2 * CW], f32)
                    tiles.append((c, (w_gd1, w_gd2), w_u, pgd, base))

                todo = pending_mm3
                pending_mm3 = []
                for hc in range(HC):
                    for c, w_gds, w_u, pgd, base in tiles:
                        w_half = w_gds[hc // 4]
                        off = (hc % 4) * 2 * CW
                        nc.tensor.matmul(
                            pgd[base:base + T, :], xt[:, hc * T:(hc + 1) * T],
                            w_half[:, off:off + 2 * CW],
                            start=(hc == 0), stop=(hc == HC - 1),
                        )
                    if todo:
                        todo.pop(0)()
                while todo:
                    todo.pop(0)()

                for c, w_gds, w_u, pgd, base in tiles:
                    emit_chunk_tail(c, w_u, pgd, base)

            for mm3 in pending_mm3:
                mm3()

            out_sb = op.tile([T, HIDDEN], f32)
            nc.vector.tensor_copy(out_sb[:], psum_out[:])
            nc.sync.dma_start(out_d.ap(), out_sb[:])

    nc.compile()
    return nc


def _get_compiled():
    global _COMPILED
    if _COMPILED is None:
        _COMPILED = _build()
    return _COMPILED


def _pack_inputs(x, gate_proj, up_proj, down_proj):
    x = np.ascontiguousarray(x, dtype=np.float32)
    # xt[p, hc*T + t] = x[t, hc*128 + p]
    xt = np.ascontiguousarray(
        x.T.reshape(HC, P, T).transpose(1, 0, 2).reshape(P, HC * T))
    eye = np.eye(T, dtype=np.float32)
    in_maps = []
    for k in range(N_CORES):
        g = np.asarray(gate_proj[k], dtype=np.float32)
        d = np.asarray(down_proj[k], dtype=np.float32)
        u = np.asarray(up_proj[k], dtype=np.float32)
        # wg4/wd4[c, p, hc, o] = g/d[c*CW + o, hc*128 + p]; interleave per hc
        wg4 = g.reshape(NCHUNK, CW, HC, P).transpose(0, 3, 2, 1)
        wd4 = d.reshape(NCHUNK, CW, HC, P).transpose(0, 3, 2, 1)
        wgd = np.concatenate([wg4, wd4], axis=3).reshape(NCHUNK, P, 2 * HC * CW)
        # Wu[c, p, f*HIDDEN + j] = u[j, c*CW + f*128 + p]
        wu = u.reshape(HIDDEN, NCHUNK, CW // P, P).transpose(1, 3, 2, 0).reshape(
            NCHUNK, P, 2 * HIDDEN)
        w = np.ascontiguousarray(
            np.concatenate([wgd, wu], axis=2), dtype=np.float32)
        in_maps.append({"xt": xt, "eye": eye, "w": w})
    return in_maps


def kernel(x, expert_indices, gate_proj, up_proj, down_proj):
    global LAST_RESULTS
    from concourse.bass_utils import run_bass_kernel_spmd

    nc = _get_compiled()
    in_maps = _pack_inputs(x, gate_proj, up_proj, down_proj)
    res = run_bass_kernel_spmd(nc, in_maps, core_ids=list(range(N_CORES)),
                               trace=TRACE)
    LAST_RESULTS = res

    expert_outs = np.stack([res.results[k]["out"] for k in range(N_CORES)])
    idx = np.asarray(expert_indices).astype(np.int64)  # [T, TOP_K]
    return expert_outs[idx, np.arange(T)[:, None], :].astype(np.float32)
